# revision 5
# baseline (speedup 1.0000x reference)
"""Trainium2 Bass kernel: PSI block (LN1 -> sigmoid-gated value -> chunked
normalized cumsum -> residual -> LN2 -> exact-gelu FFN -> residual).

Sharding: 32768 tokens split into 8 contiguous 4096-token shards (chunk- and
batch-boundary aligned), one per NeuronCore; dim-sized weights replicated.

Per core, two passes:
  pass A (32 token-tiles of 128): LN1 stats (ACT Square accum + DVE reduce,
    Newton-iteration rsqrt on DVE to stay on a single ACT table), z.T built by
    matmuls against diag(rstd), bf16 gate/value matmuls, sigmoid, chunked
    cumsum via block-triangular matmul, mem = csum(g*v)/(csum(g)+1e-6),
    x2 = x + mem -> DRAM scratch, LN2 stats stored for pass B.
  pass B (8 macros of 512 tokens): h2.T via PE transposes, FFN1 (bf16,
    output transposed [f, tok]), exact gelu via Erf, FFN2 accumulating into
    transposed PSUM out with fp32 x2-residual transposes folded into the same
    accumulation, drain [dout, tok] -> DRAM; host transposes back.
"""

import sys

sys.path.insert(0, "/opt/trn_rl_repo")

import numpy as np
import ml_dtypes
from contextlib import ExitStack

B, S, D, CHUNK = 4, 8192, 768, 64
NCORES = 8
TOTAL = B * S              # 32768 tokens
TPC = TOTAL // NCORES      # 4096 tokens per core
KD = D // 128              # 6 k-blocks over D
H = 4 * D                  # 3072 FFN hidden
KH = H // 128              # 24 k-blocks over H
MACRO = 512                # pass-B token macro
INV_SQRT2 = 0.7071067811865476


def build(T=TPC, erf_ok=True, gbias=False, vbias=False, f1bias=False, f2bias=False):
    import concourse.bass as bass
    import concourse.bacc as bacc
    import concourse.tile as tile
    from concourse import mybir

    F32 = mybir.dt.float32
    BF16 = mybir.dt.bfloat16
    I32 = mybir.dt.int32
    AF = mybir.ActivationFunctionType
    ALU = mybir.AluOpType
    PSUM = bass.MemorySpace.PSUM
    DRAM = bass.MemorySpace.DRAM
    ts = bass.ts

    NT = T // 128
    NM = T // MACRO
    NS = MACRO // 128
    any_bias = gbias or vbias or f1bias or f2bias

    nc = bacc.Bacc(None, target_bir_lowering=False, debug=False)

    x_d = nc.dram_tensor("x", [T, D], F32, kind="ExternalInput")
    wg_d = nc.dram_tensor("wg", [KD, 128, D], BF16, kind="ExternalInput")
    wv_d = nc.dram_tensor("wv", [KD, 128, D], BF16, kind="ExternalInput")
    w1_d = nc.dram_tensor("w1", [KD, 128, H], BF16, kind="ExternalInput")
    w2_d = nc.dram_tensor("w2", [KH, 128, D], BF16, kind="ExternalInput")
    u_d = nc.dram_tensor("u", [128, 128], BF16, kind="ExternalInput")
    idb_d = nc.dram_tensor("idb", [128, 128], BF16, kind="ExternalInput")
    idf_d = nc.dram_tensor("idf", [128, 128], F32, kind="ExternalInput")
    bg_d = nc.dram_tensor("bg", [1, D], BF16, kind="ExternalInput") if gbias else None
    bv_d = nc.dram_tensor("bv", [1, D], BF16, kind="ExternalInput") if vbias else None
    b1_d = nc.dram_tensor("b1", [1, H], BF16, kind="ExternalInput") if f1bias else None
    b2_d = nc.dram_tensor("b2", [1, D], BF16, kind="ExternalInput") if f2bias else None
    outT_d = nc.dram_tensor("outT", [D, T], F32, kind="ExternalOutput")

    with tile.TileContext(nc) as tc, ExitStack() as ctx:
        dram = ctx.enter_context(tc.tile_pool(name="dram", bufs=1, space=DRAM))
        x2_d = dram.tile([T, D], F32, name="x2scratch")

        const = ctx.enter_context(tc.tile_pool(name="const", bufs=1))
        wg_sb = const.tile([128, KD, D], BF16, tag="wg")
        wv_sb = const.tile([128, KD, D], BF16, tag="wv")
        w1_sb = const.tile([128, KD, H], BF16, tag="w1")
        for k in range(KD):
            nc.sync.dma_start(wg_sb[:, k, :], wg_d[k])
            nc.sync.dma_start(wv_sb[:, k, :], wv_d[k])
            nc.sync.dma_start(w1_sb[:, k, :], w1_d[k])
        w2_sb = const.tile([128, KH, D], BF16, tag="w2")
        for k in range(KH):
            nc.sync.dma_start(w2_sb[:, k, :], w2_d[k])
        u_sb = const.tile([128, 128], BF16, tag="u")
        nc.sync.dma_start(u_sb[:], u_d[:])
        idb_sb = const.tile([128, 128], BF16, tag="idb")
        nc.sync.dma_start(idb_sb[:], idb_d[:])
        idf_sb = const.tile([128, 128], F32, tag="idf")
        nc.sync.dma_start(idf_sb[:], idf_d[:])
        rstd2_all = const.tile([128, NT], F32, tag="rstd2")
        nmr2_all = const.tile([128, NT], F32, tag="nmr2")
        if gbias:
            bg_sb = const.tile([1, D], BF16, tag="bg")
            nc.sync.dma_start(bg_sb[:], bg_d[:])
        if vbias:
            bv_sb = const.tile([1, D], BF16, tag="bv")
            nc.sync.dma_start(bv_sb[:], bv_d[:])
        if f1bias:
            b1_sb = const.tile([1, H], BF16, tag="b1")
            nc.sync.dma_start(b1_sb[:], b1_d[:])
        if f2bias:
            b2_sb = const.tile([1, D], BF16, tag="b2")
            nc.sync.dma_start(b2_sb[:], b2_d[:])
        if any_bias:
            ones_sb = const.tile([1, MACRO], BF16, tag="ones")
            nc.vector.memset(ones_sb[:], 1.0)

        def ln_stats(pool, tag, src):
            """Row stats of src [128, D] f32: returns (nmu, v) = (-mean, var+eps)."""
            sqscr = pool.tile([128, D], BF16, tag="sqscr", bufs=2, name="sqscr")
            sqs = pool.tile([128, 1], F32, tag=tag + "_sqs", bufs=2, name="sqs")
            nc.scalar.activation(sqscr[:], src[:], AF.Square, accum_out=sqs[:])
            xs = pool.tile([128, 1], F32, tag=tag + "_xs", bufs=2, name="xs")
            nc.vector.tensor_reduce(xs[:], src[:], mybir.AxisListType.X, ALU.add)
            nmu = pool.tile([128, 1], F32, tag=tag + "_nmu", bufs=2, name="nmu")
            nc.vector.tensor_scalar(nmu[:], xs[:], -1.0 / D, None, op0=ALU.mult)
            v = pool.tile([128, 1], F32, tag=tag + "_v", bufs=2, name="v")
            nc.vector.tensor_scalar(v[:], sqs[:], 1.0 / D, 1e-5, op0=ALU.mult, op1=ALU.add)
            m2 = pool.tile([128, 1], F32, tag=tag + "_m2", bufs=2, name="m2")
            nc.vector.tensor_mul(m2[:], nmu[:], nmu[:])
            nc.vector.tensor_sub(v[:], v[:], m2[:])
            return nmu, v

        def newton_rsqrt(pool, tag, v, out_ap=None):
            """y ~ rsqrt(v) for v [128,1] f32 > 0; quake seed + 2 NR iters on DVE."""
            y = pool.tile([128, 1], F32, tag=tag + "_y", bufs=2, name="y")
            a = pool.tile([128, 1], F32, tag=tag + "_a", bufs=2, name="a")
            nc.vector.tensor_scalar(
                y[:].bitcast(I32), v[:].bitcast(I32), 1, -1,
                op0=ALU.logical_shift_right, op1=ALU.bitwise_xor,
            )
            nc.vector.tensor_scalar(
                y[:].bitcast(I32), y[:].bitcast(I32), 0x5F3759E0, None, op0=ALU.add
            )
            for it in range(2):
                nc.vector.tensor_mul(a[:], y[:], y[:])
                nc.vector.tensor_mul(a[:], a[:], v[:])
                nc.vector.tensor_scalar(a[:], a[:], -0.5, 1.5, op0=ALU.mult, op1=ALU.add)
                dst = out_ap if (it == 1 and out_ap is not None) else y[:]
                nc.vector.tensor_mul(dst, y[:], a[:])
            return y

        # ---------------- pass A ----------------
        with tc.tile_pool(name="pa", bufs=1) as pa, \
                tc.tile_pool(name="psa", bufs=1, space=PSUM) as psa:
            for t in range(NT):
                tok0 = 128 * t
                x_sb = pa.tile([128, D], F32, tag="x", bufs=3, name="x_sb")
                nc.sync.dma_start(x_sb[:], x_d[tok0:tok0 + 128, :])
                nmu, v = ln_stats(pa, "s1", x_sb)
                rstd = newton_rsqrt(pa, "n1", v)
                # centered x in bf16; rstd folded into the transpose below
                hu = pa.tile([128, D], BF16, tag="hu", bufs=2, name="hu")
                nc.scalar.activation(hu[:], x_sb[:], AF.Identity, bias=nmu[:])
                dg = pa.tile([128, 128], BF16, tag="dg", bufs=2, name="dg")
                nc.vector.tensor_scalar(dg[:], idb_sb[:], rstd[:], None, op0=ALU.mult)
                # z.T blocks: [d_blk, tok] = hu_blk.T @ diag(rstd)
                lnT_ps = psa.tile([128, KD, 128], F32, tag="lnT", bufs=2,
                                  padded_shape=[128, 8, 128], name="lnT_ps")
                for k in range(KD):
                    nc.tensor.matmul(lnT_ps[:, k, :], hu[:, ts(k, 128)], dg[:],
                                     start=True, stop=True)
                lnT = pa.tile([128, KD, 128], BF16, tag="lnT", bufs=2, name="lnT")
                nc.scalar.copy(lnT[:], lnT_ps[:])
                # gate / value pre-activations [tok, D] in two 384-halves
                pg = psa.tile([128, 2, 512], F32, tag="pg", bufs=1, name="pg")
                pv = psa.tile([128, 2, 512], F32, tag="pv", bufs=1, name="pv")
                for hh in range(2):
                    c0 = 384 * hh
                    mm = [(lnT[:, k, :], wg_sb[:, k, c0:c0 + 384]) for k in range(KD)]
                    if gbias:
                        mm.append((ones_sb[0:1, 0:128], bg_sb[0:1, c0:c0 + 384]))
                    for i, (l, r) in enumerate(mm):
                        nc.tensor.matmul(pg[:, hh, 0:384], l, r,
                                         start=(i == 0), stop=(i == len(mm) - 1))
                    mm = [(lnT[:, k, :], wv_sb[:, k, c0:c0 + 384]) for k in range(KD)]
                    if vbias:
                        mm.append((ones_sb[0:1, 0:128], bv_sb[0:1, c0:c0 + 384]))
                    for i, (l, r) in enumerate(mm):
                        nc.tensor.matmul(pv[:, hh, 0:384], l, r,
                                         start=(i == 0), stop=(i == len(mm) - 1))
                g_sb = pa.tile([128, 2, 384], BF16, tag="g", bufs=2, name="g_sb")
                nc.scalar.activation(g_sb[:], pg[:, :, 0:384], AF.Sigmoid)
                gv_sb = pa.tile([128, 2, 384], BF16, tag="gv", bufs=2, name="gv_sb")
                nc.vector.tensor_mul(gv_sb[:], g_sb[:], pv[:, :, 0:384])
                # chunked cumsum along tokens (partition dim) via triangular matmul
                csv = psa.tile([128, 2, 512], F32, tag="pg", bufs=1, name="csv")
                csc = psa.tile([128, 2, 512], F32, tag="pv", bufs=1, name="csc")
                for hh in range(2):
                    nc.tensor.matmul(csv[:, hh, 0:384], u_sb[:], gv_sb[:, hh, :],
                                     start=True, stop=True)
                    nc.tensor.matmul(csc[:, hh, 0:384], u_sb[:], g_sb[:, hh, :],
                                     start=True, stop=True)
                den = pa.tile([128, D], F32, tag="den", bufs=2, name="den")
                rcp = pa.tile([128, D], F32, tag="rcp", bufs=2, name="rcp")
                mem = pa.tile([128, D], F32, tag="mem", bufs=2, name="mem")
                x2 = pa.tile([128, D], F32, tag="x2", bufs=3, name="x2")
                for hh in range(2):
                    sl = slice(384 * hh, 384 * hh + 384)
                    nc.vector.tensor_scalar(den[:, sl], csc[:, hh, 0:384], 1e-6, None,
                                            op0=ALU.add)
                    nc.vector.reciprocal_approx_fast(rcp[:, sl], den[:, sl])
                    nc.vector.tensor_mul(mem[:, sl], rcp[:, sl], csv[:, hh, 0:384])
                    nc.vector.tensor_add(x2[:, sl], x_sb[:, sl], mem[:, sl])
                nc.sync.dma_start(x2_d[tok0:tok0 + 128, :], x2[:])
                nmu2, v2 = ln_stats(pa, "s2", x2)
                newton_rsqrt(pa, "n2", v2, out_ap=rstd2_all[:, t:t + 1])
                nc.vector.tensor_mul(nmr2_all[:, t:t + 1], nmu2[:], rstd2_all[:, t:t + 1])

        # ---------------- pass B ----------------
        with tc.tile_pool(name="pb", bufs=1) as pb, \
                tc.tile_pool(name="psb", bufs=1, space=PSUM) as psb:
            for m in range(NM):
                tok0 = MACRO * m
                outT_ps = psb.tile([128, KD, MACRO], F32, tag="outT", bufs=1,
                                   name="outT_ps")
                h2T = pb.tile([128, KD, MACRO], BF16, tag="h2T", bufs=1, name="h2T")
                x2s_list = []
                for s in range(NS):
                    tm = m * NS + s
                    x2s = pb.tile([128, D], F32, tag="x2s", bufs=NS + 1, name="x2s")
                    x2s_list.append(x2s)
                    nc.sync.dma_start(x2s[:], x2_d[tok0 + 128 * s:tok0 + 128 * (s + 1), :])
                    h2s = pb.tile([128, D], BF16, tag="h2s", bufs=2, name="h2s")
                    nc.vector.tensor_scalar(h2s[:], x2s[:], rstd2_all[:, tm:tm + 1],
                                            nmr2_all[:, tm:tm + 1],
                                            op0=ALU.mult, op1=ALU.add)
                    tps = psb.tile([128, KD, 128], BF16, tag="pt", bufs=2,
                                   padded_shape=[128, 8, 128], name="tps")
                    for k in range(KD):
                        nc.tensor.transpose(tps[:, k, :], h2s[:, ts(k, 128)], idb_sb[:])
                    nc.scalar.copy(h2T[:, :, 128 * s:128 * (s + 1)], tps[:])
                for f in range(KH):
                    pT = psb.tile([128, MACRO], F32, tag="pt", bufs=2, name="pT")
                    mm = [(w1_sb[:, k, 128 * f:128 * (f + 1)], h2T[:, k, :])
                          for k in range(KD)]
                    if f1bias:
                        mm.append((b1_sb[0:1, 128 * f:128 * (f + 1)],
                                   ones_sb[0:1, 0:MACRO]))
                    for i, (l, r) in enumerate(mm):
                        nc.tensor.matmul(pT[:], l, r,
                                         start=(i == 0), stop=(i == len(mm) - 1))
                    e_sb = pb.tile([128, MACRO], BF16, tag="e", bufs=2, name="e_sb")
                    nc.scalar.activation(e_sb[:], pT[:],
                                         AF.Erf if erf_ok else AF.Tanh, scale=INV_SQRT2)
                    uT = pb.tile([128, MACRO], BF16, tag="uT", bufs=3, name="uT")
                    nc.vector.scalar_tensor_tensor(uT[:], e_sb[:], 1.0, pT[:],
                                                   op0=ALU.add, op1=ALU.mult)
                    # f == 0 opens each bank's group full-width; residual/bias
                    # transposes then accumulate into the open group.
                    for m2 in range(KD):
                        nc.tensor.matmul(outT_ps[:, m2, :],
                                         w2_sb[:, f, 128 * m2:128 * (m2 + 1)], uT[:],
                                         start=(f == 0), stop=(f == KH - 1))
                    if f == 0:
                        for s in range(NS):
                            for m2 in range(KD):
                                nc.tensor.matmul(
                                    outT_ps[:, m2, 128 * s:128 * (s + 1)],
                                    x2s_list[s][:, ts(m2, 128)], idf_sb[:],
                                    start=False, stop=False)
                        if f2bias:
                            for m2 in range(KD):
                                nc.tensor.matmul(outT_ps[:, m2, :],
                                                 b2_sb[0:1, 128 * m2:128 * (m2 + 1)],
                                                 ones_sb[0:1, 0:MACRO],
                                                 start=False, stop=False)
                for m2 in range(KD):
                    osb = pb.tile([128, MACRO], F32, tag="osb", bufs=2, name="osb")
                    nc.scalar.copy(osb[:], outT_ps[:, m2, :])
                    nc.sync.dma_start(outT_d[128 * m2:128 * (m2 + 1), tok0:tok0 + MACRO],
                                      osb[:])

    nc.compile()
    return nc


def _fold(inputs):
    f32 = np.float32
    bf16 = ml_dtypes.bfloat16
    n1w = np.asarray(inputs["norm1_w"], f32)
    n1b = np.asarray(inputs["norm1_b"], f32)
    n2w = np.asarray(inputs["norm2_w"], f32)
    n2b = np.asarray(inputs["norm2_b"], f32)
    gW = np.asarray(inputs["gate_W"], f32)
    gb = np.asarray(inputs["gate_b"], f32)
    vW = np.asarray(inputs["value_W"], f32)
    vb = np.asarray(inputs["value_b"], f32)
    W1 = np.asarray(inputs["ffn_W1"], f32)
    b1 = np.asarray(inputs["ffn_b1"], f32)
    W2 = np.asarray(inputs["ffn_W2"], f32)
    b2 = np.asarray(inputs["ffn_b2"], f32)

    bg = (n1b @ gW + gb).astype(bf16).reshape(1, D)
    bv = (n1b @ vW + vb).astype(bf16).reshape(1, D)
    b1f = (n2b @ W1 + b1).astype(bf16).reshape(1, H)
    b2f = b2.astype(bf16).reshape(1, D)
    flags = (bool(bg.any()), bool(bv.any()), bool(b1f.any()), bool(b2f.any()))

    tri = np.triu(np.ones((CHUNK, CHUNK), f32))
    u = np.zeros((128, 128), f32)
    for c in range(128 // CHUNK):
        u[c * CHUNK:(c + 1) * CHUNK, c * CHUNK:(c + 1) * CHUNK] = tri

    arrs = {
        "wg": np.ascontiguousarray((n1w[:, None] * gW).reshape(KD, 128, D).astype(bf16)),
        "wv": np.ascontiguousarray((n1w[:, None] * vW).reshape(KD, 128, D).astype(bf16)),
        "w1": np.ascontiguousarray((n2w[:, None] * W1).reshape(KD, 128, H).astype(bf16)),
        "w2": np.ascontiguousarray((0.5 * W2).reshape(KH, 128, D).astype(bf16)),
        "u": u.astype(bf16),
        "idb": np.eye(128, dtype=bf16),
        "idf": np.eye(128, dtype=f32),
    }
    if flags[0]:
        arrs["bg"] = bg
    if flags[1]:
        arrs["bv"] = bv
    if flags[2]:
        arrs["b1"] = b1f
    if flags[3]:
        arrs["b2"] = b2f
    return arrs, flags


_CACHE: dict = {}


def _get_exec(flags):
    """Build (once) the Bass module and a cached jitted PJRT executable."""
    if _CACHE.get("flags") == flags:
        return _CACHE
    import jax
    from concourse import bass2jax
    from concourse import mybir
    from concourse.bass2jax import (
        Mesh, PartitionSpec, shard_map, _bass_exec_p, install_neuronx_cc_hook,
        partition_id_tensor,
    )

    nc = build(TPC, True, *flags)
    install_neuronx_cc_hook()
    assert nc.dbg_addr is None
    partition_name = nc.partition_id_tensor.name if nc.partition_id_tensor else None

    in_names, out_names, out_avals, zero_outs = [], [], [], []
    for alloc in nc.m.functions[0].allocations:
        if not isinstance(alloc, mybir.MemoryLocationSet):
            continue
        name = alloc.memorylocations[0].name
        if alloc.kind == "ExternalInput":
            if name != partition_name:
                in_names.append(name)
        elif alloc.kind == "ExternalOutput":
            shape = tuple(alloc.tensor_shape)
            dtype = mybir.dt.np(alloc.dtype)
            out_names.append(name)
            out_avals.append(jax.core.ShapedArray(shape, dtype))
            zero_outs.append(np.zeros(shape, dtype))
    n_params = len(in_names)
    n_outs = len(out_avals)
    all_names = in_names + out_names
    if partition_name is not None:
        all_names = all_names + [partition_name]
    donate = tuple(range(n_params, n_params + n_outs))

    def _body(*args):
        operands = list(args)
        if partition_name is not None:
            operands.append(partition_id_tensor())
        outs = _bass_exec_p.bind(
            *operands,
            out_avals=tuple(out_avals),
            in_names=tuple(all_names),
            out_names=tuple(out_names),
            lowering_input_output_aliases=(),
            sim_require_finite=True,
            sim_require_nnan=True,
            nc=nc,
        )
        return tuple(outs)

    devices = jax.devices()[:NCORES]
    assert len(devices) == NCORES
    mesh = Mesh(np.asarray(devices), ("core",))
    sharded = jax.jit(
        shard_map(_body, mesh=mesh, in_specs=(PartitionSpec("core"),) * (n_params + n_outs),
                  out_specs=(PartitionSpec("core"),) * n_outs, check_rep=False),
        donate_argnums=donate, keep_unused=True,
    )
    _CACHE.clear()
    _CACHE.update(
        flags=flags, nc=nc, sharded=sharded, in_names=in_names,
        out_names=out_names, out_avals=out_avals, zero_outs=zero_outs, mesh=mesh,
    )
    return _CACHE


def _run(arrs, flags, x_flat):
    st = _get_exec(flags)
    concat_in = []
    for name in st["in_names"]:
        if name == "x":
            concat_in.append(np.ascontiguousarray(x_flat))
        else:
            a = arrs[name]
            concat_in.append(np.concatenate([a] * NCORES, axis=0))
    concat_zeros = [
        np.zeros((NCORES * z.shape[0], *z.shape[1:]), z.dtype) for z in st["zero_outs"]
    ]
    out_arrs = st["sharded"](*concat_in, *concat_zeros)
    i = st["out_names"].index("outT")
    o = np.asarray(out_arrs[i]).reshape(NCORES, D, TPC)
    return o


def kernel(**inputs):
    x = np.asarray(inputs["x"], np.float32).reshape(TOTAL, D)
    arrs, flags = _fold(inputs)
    try:
        o = _run(arrs, flags, x)
        parts = [o[c].T for c in range(NCORES)]
    except Exception:
        from concourse.bass_utils import run_bass_kernel_spmd
        if _CACHE.get("flags") != flags or "nc" not in _CACHE:
            _CACHE.clear()
            _CACHE["nc"] = build(TPC, True, *flags)
            _CACHE["flags"] = flags
        in_maps = [
            {**arrs, "x": np.ascontiguousarray(x[c * TPC:(c + 1) * TPC])}
            for c in range(NCORES)
        ]
        res = run_bass_kernel_spmd(_CACHE["nc"], in_maps, list(range(NCORES)),
                                   trace=False)
        parts = [np.asarray(res.results[c]["outT"]).T for c in range(NCORES)]
    return np.concatenate(parts, axis=0).reshape(B, S, D).astype(np.float32)
